# revision 21
# baseline (speedup 1.0000x reference)
"""MLA q/kv projection kernel for Trainium2, 8 NeuronCores, SPMD data-parallel
over the token dimension (512 tokens per core).

Per-core pipeline (v2 — transposed-cq restructure):
  kvmm:  kv[512t, 512] (token-major, x stationary) and kropeT[64, 512t]
         (wkv rope cols stationary) streamed over k; runs first as PE warmup
         while DMA ramps.  kv rmsnorm*gamma + rope token-major, stored.
  cqmm:  cqT[1536, 512t] = wq_a.T @ x.T in 3 phases of 4x128 col-blocks
         (weights stationary, tokens moving) -> PSUM holds cq TRANSPOSED,
         which is exactly mm2's stationary layout: no transposes at all.
         cq is NOT normalized here; evictions cast to bf16 cqT tiles and
         Act squares each tile; 1-row ones-matmuls reduce the squares over
         partitions into ssq[tok,1] per m (pipelined one phase late so PE
         never waits on Act).
  s4:    1/sqrt(ssq/L + eps) per token = the rmsnorm scale, applied at mm2
         eviction (rope is per-token linear, so post-scaling is exact).
         gamma_cq is folded into wq_b on the host.
  mm2:   q[512t, 24576] = cq.T @ wb in ORIGINAL head order, 64 n-tiles of
         384 cols (2 heads each): eviction does nope-cols scalar-mul by s4
         and rope with s4-prescaled cos/sin, then one fully contiguous
         store per (n, m) tile (1536B runs).

Host-side prep: shard+transpose token_x; pre-tile wkv/wq_a/wq_b into
per-DMA-contiguous layouts; fold gamma_cq into wq_b; pack cos/sin as
[c1|c1|c2|c2|s1|s1|s2|s2]x32 so 2-head-fused rope slices line up.
"""

import os

import numpy as np

import concourse.bass as bass
import concourse.tile as tile
from concourse import mybir
from concourse.bass_utils import run_bass_kernel_spmd
from concourse.masks import make_identity
from concourse.vector_clock import ScopedClock, VectorClock

F32 = mybir.dt.float32
BF16 = mybir.dt.bfloat16

N_CORES = 8
T = 4096
TC = T // N_CORES           # 512 tokens per core
MT = TC // 128              # 4 token tiles
H = 7168
KH = H // 128               # 56 contraction tiles for mm1
KH2 = KH // 2               # 28 paired-row loads
L = 1536                    # q latent
KL = L // 128               # 12 contraction tiles for mm2
KL2 = KL // 2
KV_RANK = 512
R = 64                      # rope dims
N_HEADS = 128
QK_NOPE = 128
DN = N_HEADS * (QK_NOPE + R)   # 24576
NW = 512                    # mm2 n-tile width (2 2/3 heads)
NT = DN // NW               # 48 n-tiles
OUTW = DN + KV_RANK + R     # 25152
EPS = 1e-6

# mm2 eviction patterns per (tile_index % 3): head-aligned 192-col periods
# sliced by 512-wide tiles.  rope: (base, nblocks); nope: list of
# (base, nblocks, width).  All blocks stride 192.
MM2_PAT = [
    {"rope": (128, 2), "nope": [(0, 3, 128)]},
    {"rope": (0, 3), "nope": [(64, 2, 128), (448, 1, 64)]},
    {"rope": (64, 3), "nope": [(0, 1, 64), (128, 2, 128)]},
]


def _blocks(ap2d, base, nb, w, stride=192):
    """3D view of a 2D AP: [partitions, nb blocks (elem stride `stride`),
    w contiguous elems] starting at free-offset `base`."""
    return bass.AP(tensor=ap2d.tensor, offset=ap2d.offset + base,
                   ap=[list(ap2d.ap[0]), [stride, nb], [1, w]])


def split_multi_waits(nc, limit=1):
    """Walrus in this toolchain accepts at most one sync-wait command per
    TPB instruction. Hoist extra waits onto single-wait NoOps inserted
    immediately before the offending instruction on the same engine."""
    skip = (mybir.InstAllEngineBarrier, mybir.InstEventSemaphore)
    for f in nc.m.functions:
        for bb in f.blocks:
            new_insts = []
            changed = False
            for inst in bb.instructions:
                si = inst.sync_info
                waits = list(si.on_wait) if si is not None and si.on_wait else []
                if len(waits) > limit and not isinstance(inst, skip):
                    for w in waits[:-limit]:
                        nop = mybir.InstNoOp(
                            name=nc.get_next_instruction_name(),
                            sync_info=mybir.SyncInfo(on_wait=[w], on_update=[]),
                            bass_nofuse=True,
                            engine=inst.engine,
                        )
                        new_insts.append(nop)
                    inst.sync_info = mybir.SyncInfo(
                        on_wait=waits[-limit:], on_update=list(si.on_update))
                    changed = True
                new_insts.append(inst)
            if changed:
                bb.instructions = new_insts
    return nc


class PatchedTC(tile.TileContext):
    """Workaround for the same walrus limit at the kernel tail: the SP Drain
    only accepts ONE sync-wait, while Tile attaches one per active processor.
    Chain single-wait drains instead."""

    def _drain_and_barrier(self, tick_clock, wait_clock):
        nc = self.nc
        gc = tick_clock.global_clock
        nprocs = len(gc)
        procs = [p for p in range(nprocs) if gc[p] > 0] or [0]
        for p in procs:
            d = nc.sync.drain()
            vc = VectorClock([0] * nprocs)
            vc.require_at_least(p, gc[p])
            wait_clock.add_sem_waits(d.ins, ScopedClock({None: vc}))
        nc.all_engine_barrier()
        assert self.sems is not None
        popped = nc._tile_sem_poison_stack.pop()
        assert popped is self._sem_poison
        nc.clear_and_free_semaphores(list(self.sems.allocated().values()))
        nc.all_engine_barrier()


def build_nc(split=True):
    reps = int(os.environ.get("MLA_REPS", "1"))
    strip = os.environ.get("MLA_STRIP", "")   # "", "evict", "store"
    wb_nodma = os.environ.get("MLA_WB_NODMA", "") == "1"   # timing experiment
    # bf16 outputs (host upcasts): same kernel time, half the store/download
    # bytes; rms-rel 3.7e-3 vs 3.3e-3 — far under the 2e-2 gate
    out_bf16 = os.environ.get("MLA_OUT_BF16", "1") == "1"
    wb_bufs = int(os.environ.get("MLA_WB_BUFS", "16"))
    wdq_bufs = int(os.environ.get("MLA_WDQ_BUFS", "6"))
    wdkv_bufs = int(os.environ.get("MLA_WDKV_BUFS", "8"))
    qout_bufs = int(os.environ.get("MLA_QOUT_BUFS", "8"))

    Sq = mybir.ActivationFunctionType.Square
    Sqrt = mybir.ActivationFunctionType.Sqrt

    out_dt = BF16 if out_bf16 else F32
    nc = bass.Bass()
    xt = nc.dram_tensor("xt", [H, TC], BF16, kind="ExternalInput")
    wdkv = nc.dram_tensor("wdkv", [KH2 * 128, 2 * 576], BF16,
                          kind="ExternalInput")
    wdq = nc.dram_tensor("wdq", [3 * KH2 * 128, 2 * 512], BF16,
                         kind="ExternalInput")
    wb = nc.dram_tensor("wb", [NT * KL2 * 128, 2 * NW], BF16,
                        kind="ExternalInput")
    cs = nc.dram_tensor("cs", [TC, 384], F32, kind="ExternalInput")
    gkv = nc.dram_tensor("gkv", [KV_RANK], F32, kind="ExternalInput")
    out = nc.dram_tensor("out", [TC, OUTW], out_dt, kind="ExternalOutput")

    out_ap = out.ap()

    with PatchedTC(nc) as tc:
        with (
            tc.tile_pool(name="consts", bufs=1) as p_const,
            tc.tile_pool(name="cs", bufs=1) as p_cs,
            tc.tile_pool(name="xt", bufs=1) as p_xt,
            tc.tile_pool(name="wdkv", bufs=wdkv_bufs) as p_wdkv,
            tc.tile_pool(name="wdq", bufs=wdq_bufs) as p_wdq,
            tc.tile_pool(name="cqt", bufs=2) as p_cqt,
            tc.tile_pool(name="sq", bufs=8) as p_sq,
            tc.tile_pool(name="kv", bufs=1) as p_kv,
            tc.tile_pool(name="krT", bufs=1) as p_krT,
            tc.tile_pool(name="stats", bufs=2) as p_stats,
            tc.tile_pool(name="tmp", bufs=2) as p_tmp,
            tc.tile_pool(name="wb", bufs=wb_bufs) as p_wb,
            tc.tile_pool(name="qout", bufs=qout_bufs) as p_qout,
            tc.tile_pool(name="psum", bufs=8, space="PSUM") as p_ps,
        ):
            # ---- constants ----
            idf = p_const.tile([64, 64], F32, tag="idf", name="idf")
            make_identity(nc, idf)
            ones1 = p_const.tile([128, 1], BF16, tag="ones1", name="ones1")
            nc.vector.memset(ones1, 1.0)
            eps_t = p_const.tile([128, 1], F32, tag="eps", name="eps_t")
            nc.vector.memset(eps_t, EPS)
            gamma_b = p_const.tile([128, KV_RANK], F32, tag="gamma",
                                   name="gamma_b")
            gamma_o = gamma_b
            if out_bf16:
                gamma_o = p_const.tile([128, KV_RANK], BF16, tag="gammao",
                                       name="gamma_o")

            for _rep in range(reps):
                # ======== phase A: kv latent (token-major) + kropeT ========
                kv_ps = [p_ps.tile([128, 512], F32, tag="ps", name="ps")
                         for _ in range(MT)]
                kr_ps = p_ps.tile([128, 512], F32, tag="ps", name="ps")
                xt_tiles = {}
                cs_sb = []
                for k2 in range(KH2):
                    xt_tiles[k2] = p_xt.tile([128, 2, TC], BF16,
                                             tag=f"xt{k2}", name=f"xt{k2}")
                    wkv_t = p_wdkv.tile([128, 2, 576], BF16, tag="wdkv",
                                        name="wkv_t")
                    if k2 == 0:
                        # split first tiles per-b so the b=0 halves (and the
                        # first matmuls) start in half the time
                        for b in range(2):
                            nc.sync.dma_start(
                                out=xt_tiles[k2][:, b, :],
                                in_=xt.ap()[k2 * 256 + b * 128:
                                            k2 * 256 + (b + 1) * 128, :])
                            nc.sync.dma_start(
                                out=wkv_t[:, b, :],
                                in_=wdkv.ap()[k2 * 128:(k2 + 1) * 128,
                                              b * 576:(b + 1) * 576])
                    else:
                        nc.sync.dma_start(
                            out=xt_tiles[k2],
                            in_=xt.ap()[k2 * 256:(k2 + 1) * 256, :]
                            .rearrange("(b p) t -> p b t", p=128))
                        nc.sync.dma_start(
                            out=wkv_t,
                            in_=wdkv.ap()[k2 * 128:(k2 + 1) * 128, :]
                            .rearrange("p (b c) -> p b c", c=576))
                    if k2 == 0:
                        # small consts after first big tiles are queued
                        g_ap = gkv.ap()
                        nc.sync.dma_start(
                            out=gamma_b,
                            in_=bass.AP(tensor=g_ap.tensor, offset=g_ap.offset,
                                        ap=[[0, 128]] + [list(p)
                                                         for p in g_ap.ap]))
                        if out_bf16:
                            nc.vector.tensor_copy(out=gamma_o, in_=gamma_b)
                        for m in range(MT):
                            t = p_cs.tile([128, 384], F32, tag=f"cs{m}",
                                          name=f"cs{m}")
                            nc.sync.dma_start(
                                out=t,
                                in_=cs.ap()[m * 128:(m + 1) * 128, :])
                            cs_sb.append(t)
                    for b in range(2):
                        k = 2 * k2 + b
                        for m in range(MT):
                            nc.tensor.matmul(
                                kv_ps[m],
                                lhsT=xt_tiles[k2][:, b,
                                                  m * 128:(m + 1) * 128],
                                rhs=wkv_t[:, b, 0:512],
                                start=(k == 0), stop=(k == KH - 1))
                        nc.tensor.matmul(
                            kr_ps[0:64, :],
                            lhsT=wkv_t[:, b, 512:576],
                            rhs=xt_tiles[k2][:, b, :],
                            start=(k == 0), stop=(k == KH - 1))

                # kv rmsnorm * gamma (Act+DVE only; PE moves on to phase B)
                kv_sb = []
                for m in range(MT):
                    kv_m = p_kv.tile([128, KV_RANK + R], out_dt, tag=f"kv{m}",
                                     name=f"kv{m}")
                    st = p_stats.tile([128, 1], F32, tag=f"st{m}",
                                      name=f"st{m}")
                    scr = p_sq.tile([128, 512], BF16, tag="sq", name="scr")
                    nc.scalar.activation(
                        out=scr, in_=kv_ps[m], func=Sq, accum_out=st)
                    nc.scalar.activation(
                        out=st, in_=st, func=Sqrt,
                        bias=eps_t, scale=1.0 / KV_RANK)
                    nc.vector.reciprocal(out=st, in_=st)
                    nc.vector.tensor_scalar_mul(
                        out=kv_m[:, 0:KV_RANK], in0=kv_ps[m], scalar1=st)
                    nc.vector.tensor_mul(
                        out=kv_m[:, 0:KV_RANK], in0=kv_m[:, 0:KV_RANK],
                        in1=gamma_o)
                    kv_sb.append(kv_m)

                # ======== phase B: cqT in 3 phases of 4 col-blocks ========
                cqT = p_cqt.tile([128, KL, TC], BF16, tag="cqt", name="cqT")
                ssq_sb = p_stats.tile([128, MT], F32, tag="ssq",
                                      name="ssq_sb")
                sq_tiles = {}

                def emit_ssq(p):
                    # 1-row ones-matmuls: ssq[tok,m] = sum_part sq[p]^2.
                    # PSUM allows one open accumulation group per bank, so
                    # per-m groups run sequentially and per-phase partials
                    # accumulate into SBUF.
                    ss_ps = p_ps.tile([128, 512], F32, tag="ps",
                                      name="ss_ps")[:, 0:MT]
                    for m in range(MT):
                        for ct in range(4):
                            nc.tensor.matmul(
                                ss_ps[:, m:m + 1],
                                lhsT=sq_tiles[p][ct][:,
                                                     m * 128:(m + 1) * 128],
                                rhs=ones1,
                                start=(ct == 0), stop=(ct == 3))
                    if p == 0:
                        nc.vector.tensor_copy(out=ssq_sb, in_=ss_ps)
                    else:
                        nc.vector.tensor_add(out=ssq_sb, in0=ssq_sb,
                                             in1=ss_ps)

                def emit_krope_tail():
                    # kropeT -> token-major (PE transpose), rope, kv store
                    krT = p_krT.tile([64, 512], F32, tag="krT", name="krT")
                    nc.vector.tensor_copy(out=krT, in_=kr_ps[0:64, :])
                    for m in range(MT):
                        tpk = p_ps.tile([128, 512], F32, tag="ps", name="ps")
                        nc.tensor.transpose(
                            tpk[:, 0:64], krT[:, m * 128:(m + 1) * 128], idf)
                        kv_m = kv_sb[m]
                        x1 = tpk[:, 0:32]
                        x2 = tpk[:, 32:64]
                        xx = tpk[:, 0:64]
                        cpair = cs_sb[m][:, 0:64]
                        s1n = cs_sb[m][:, 192:224]
                        s2 = cs_sb[m][:, 288:320]
                        ta = p_tmp.tile([128, 192], F32, tag="ta", name="ta")
                        tb = p_tmp.tile([128, 192], F32, tag="tb", name="tb")
                        nc.vector.tensor_mul(out=ta[:, 0:64], in0=xx,
                                             in1=cpair)
                        nc.vector.tensor_mul(out=tb[:, 0:32], in0=x2,
                                             in1=s1n)
                        nc.vector.tensor_mul(out=tb[:, 32:64], in0=x1,
                                             in1=s2)
                        nc.vector.tensor_add(
                            out=kv_m[:, KV_RANK:KV_RANK + R],
                            in0=ta[:, 0:64], in1=tb[:, 0:64])
                        nc.sync.dma_start(
                            out=out_ap[m * 128:(m + 1) * 128, DN:OUTW],
                            in_=kv_m)

                for p in range(3):
                    cb_ps = [p_ps.tile([128, 512], F32, tag="ps", name="ps")
                             for _ in range(4)]
                    for k2 in range(KH2):
                        wdq_t = p_wdq.tile([128, 2, 512], BF16, tag="wdq",
                                           name="wdq_t")
                        nc.sync.dma_start(
                            out=wdq_t,
                            in_=wdq.ap()[(p * KH2 + k2) * 128:
                                         (p * KH2 + k2 + 1) * 128, :]
                            .rearrange("q (b c) -> q b c", c=512))
                        for b in range(2):
                            k = 2 * k2 + b
                            for cb in range(4):
                                nc.tensor.matmul(
                                    cb_ps[cb],
                                    lhsT=wdq_t[:, b,
                                               cb * 128:(cb + 1) * 128],
                                    rhs=xt_tiles[k2][:, b, :],
                                    start=(k == 0), stop=(k == KH - 1))
                        if p == 0 and k2 == 1:
                            emit_krope_tail()
                        if p > 0 and k2 == 2:
                            emit_ssq(p - 1)
                    sq_tiles[p] = []
                    for cb in range(4):
                        kq = p * 4 + cb
                        nc.vector.tensor_copy(
                            out=cqT[:, kq, :], in_=cb_ps[cb])
                        sq = p_sq.tile([128, 512], BF16, tag="sq", name="sq")
                        nc.scalar.activation(out=sq, in_=cb_ps[cb], func=Sq)
                        sq_tiles[p].append(sq)

                # ======== mm2: q = cq.T @ wb, original head order ========
                s4 = p_stats.tile([128, MT], F32, tag="s4", name="s4")
                cs2s = []
                for n in range(NT):
                    pat = MM2_PAT[n % 3]
                    q_ps = [p_ps.tile([128, 512], F32, tag="ps", name="ps")
                            for _ in range(MT)]
                    for k2 in range(KL2):
                        wb_t = p_wb.tile([128, 2, NW], BF16, tag="wb",
                                         name="wb_t")
                        if not wb_nodma or n < NT // 8:
                            nc.sync.dma_start(
                                out=wb_t,
                                in_=wb.ap()[(n * KL2 + k2) * 128:
                                            (n * KL2 + k2 + 1) * 128, :]
                                .rearrange("q (b c) -> q b c", c=NW))
                        if n == 0 and k2 == 2:
                            emit_ssq(2)
                            # s4 = 1/sqrt(ssq/L + eps); prescale cos/sin
                            nc.scalar.activation(
                                out=s4, in_=ssq_sb, func=Sqrt,
                                bias=eps_t, scale=1.0 / L)
                            nc.vector.reciprocal(out=s4, in_=s4)
                            for mm in range(MT):
                                css = p_cs.tile([128, 384], F32,
                                                tag=f"css{mm}",
                                                name=f"css{mm}", bufs=2)
                                nc.vector.tensor_scalar_mul(
                                    out=css, in0=cs_sb[mm],
                                    scalar1=s4[:, mm:mm + 1])
                                cs2s.append(css)
                        for b in range(2):
                            k = 2 * k2 + b
                            for m in range(MT):
                                nc.tensor.matmul(
                                    q_ps[m],
                                    lhsT=cqT[:, k, m * 128:(m + 1) * 128],
                                    rhs=wb_t[:, b, :],
                                    start=(k == 0), stop=(k == KL - 1))
                    for m in range(MT):
                        if strip == "evict":
                            continue
                        qo = p_qout.tile([128, NW], out_dt, tag="q", name="qo")
                        qp = q_ps[m]
                        for base, nb, w in pat["nope"]:
                            nc.vector.tensor_scalar_mul(
                                out=_blocks(qo, base, nb, w),
                                in0=_blocks(qp, base, nb, w),
                                scalar1=s4[:, m:m + 1])
                        # 4-op fused rope: out = x*[c1|c2] + [x2|x1]*[-s1|s2]
                        rb, nr = pat["rope"]
                        x = _blocks(qp, rb, nr, 64)
                        x1 = _blocks(qp, rb, nr, 32)
                        x2 = _blocks(qp, rb + 32, nr, 32)
                        o = _blocks(qo, rb, nr, 64)
                        csm = cs2s[m]
                        cpair = _blocks(csm, 0, nr, 64, stride=64)
                        s1n = _blocks(csm, 192, nr, 32, stride=32)
                        s2 = _blocks(csm, 288, nr, 32, stride=32)
                        ta = p_tmp.tile([128, 192], F32, tag="ta", name="ta")
                        tb = p_tmp.tile([128, 192], F32, tag="tb", name="tb")
                        va = _blocks(ta, 0, nr, 64, stride=64)
                        vb_lo = _blocks(tb, 0, nr, 32, stride=64)
                        vb_hi = _blocks(tb, 32, nr, 32, stride=64)
                        vb = _blocks(tb, 0, nr, 64, stride=64)
                        nc.vector.tensor_mul(out=va, in0=x, in1=cpair)
                        nc.vector.tensor_mul(out=vb_lo, in0=x2, in1=s1n)
                        nc.vector.tensor_mul(out=vb_hi, in0=x1, in1=s2)
                        nc.vector.tensor_add(out=o, in0=va, in1=vb)
                        if strip != "store":
                            nc.sync.dma_start(
                                out=out_ap[m * 128:(m + 1) * 128,
                                           n * NW:(n + 1) * NW],
                                in_=qo)
    if split:
        split_multi_waits(nc)
    return nc


def build_nc_tp(split=True):
    """Tensor-parallel mm2: AllGather normalized-latent cq across cores,
    each core computes q for ALL tokens x its 16 heads (1/8 of wq_b)."""
    reps = int(os.environ.get("MLA_REPS", "1"))
    out_bf16 = os.environ.get("MLA_OUT_BF16", "1") == "1"
    wb_bufs = int(os.environ.get("MLA_WB_BUFS", "12"))
    wdq_bufs = int(os.environ.get("MLA_WDQ_BUFS", "6"))
    wdkv_bufs = int(os.environ.get("MLA_WDKV_BUFS", "8"))
    qout_bufs = int(os.environ.get("MLA_QOUT_BUFS", "8"))

    NTL = NT // N_CORES         # 6 local n-tiles
    MTG = T // 128              # 32 global token tiles
    out_dt = BF16 if out_bf16 else F32

    Sq = mybir.ActivationFunctionType.Square
    Sqrt = mybir.ActivationFunctionType.Sqrt

    nc = bass.Bass(num_devices=N_CORES)
    xt = nc.dram_tensor("xt", [H, TC], BF16, kind="ExternalInput")
    wdkv = nc.dram_tensor("wdkv", [KH2 * 128, 2 * 576], BF16,
                          kind="ExternalInput")
    wdq = nc.dram_tensor("wdq", [3 * KH2 * 128, 2 * 512], BF16,
                         kind="ExternalInput")
    wb = nc.dram_tensor("wb", [NTL * KL2 * 128, 2 * NW], BF16,
                        kind="ExternalInput")
    cs = nc.dram_tensor("cs", [T, 384], F32, kind="ExternalInput")
    cs_loc = nc.dram_tensor("cs_loc", [TC, 384], F32, kind="ExternalInput")
    gkv = nc.dram_tensor("gkv", [KV_RANK], F32, kind="ExternalInput")
    qout = nc.dram_tensor("qout", [T, NTL * NW], out_dt,
                          kind="ExternalOutput")
    kvout = nc.dram_tensor("kvout", [TC, KV_RANK + R], out_dt,
                           kind="ExternalOutput")

    qout_ap = qout.ap()

    with PatchedTC(nc) as tc:
        with (
            tc.tile_pool(name="consts", bufs=1) as p_const,
            tc.tile_pool(name="cs", bufs=1) as p_cs,
            tc.tile_pool(name="kv", bufs=1) as p_kv,
            tc.tile_pool(name="krT", bufs=1) as p_krT,
            tc.tile_pool(name="stats", bufs=2) as p_stats,
            tc.tile_pool(name="tmp", bufs=2) as p_tmp,
            tc.tile_pool(name="qout", bufs=qout_bufs) as p_qout,
            tc.tile_pool(name="psum", bufs=8, space="PSUM") as p_ps,
            tc.tile_pool(name="dram", bufs=1, space="DRAM") as p_dram,
        ):
            # ---- constants ----
            idf = p_const.tile([64, 64], F32, tag="idf", name="idf")
            make_identity(nc, idf)
            ones1 = p_const.tile([128, 1], BF16, tag="ones1", name="ones1")
            nc.vector.memset(ones1, 1.0)
            eps_t = p_const.tile([128, 1], F32, tag="eps", name="eps_t")
            nc.vector.memset(eps_t, EPS)
            gamma_b = p_const.tile([128, KV_RANK], F32, tag="gamma",
                                   name="gamma_b")
            gamma_o = gamma_b
            if out_bf16:
                gamma_o = p_const.tile([128, KV_RANK], BF16, tag="gammao",
                                       name="gamma_o")
            cs_all = p_cs.tile([128, MTG, 384], F32, tag="cs", name="cs_all")
            cs_loc_t = p_cs.tile([128, MT, 384], F32, tag="csl",
                                 name="cs_loc_t")
            cs_loc_sb = [cs_loc_t[:, m, :] for m in range(MT)]

            for _rep in range(reps):
                # per-rep DRAM bounce buffers: a Shared tensor may only
                # have a single writing instruction in the NEFF
                s4b_in = p_dram.tile([128, MT], F32, tag=f"s4i{_rep}",
                                     name="s4b_in")
                s4b_out = p_dram.tile([N_CORES * 128, MT], F32,
                                      tag=f"s4o{_rep}", name="s4b_out",
                                      addr_space="Shared")
                cg_in = p_dram.tile([128, KL * TC], BF16, tag=f"cgi{_rep}",
                                    name="cg_in")
                cg_out = p_dram.tile([N_CORES * 128, KL * TC], BF16,
                                     tag=f"cgo{_rep}", name="cg_out",
                                     addr_space="Shared")
                with (
                    tc.tile_pool(name="xt", bufs=1) as p_xt,
                    tc.tile_pool(name="wdkv", bufs=wdkv_bufs) as p_wdkv,
                    tc.tile_pool(name="wdq", bufs=wdq_bufs) as p_wdq,
                    tc.tile_pool(name="cqt", bufs=1) as p_cqt,
                    tc.tile_pool(name="sq", bufs=8) as p_sq,
                ):
                    # ==== cq phases: cqT[1536, 512t] + ssq, xt streams in ====
                    cqT = p_cqt.tile([128, KL, TC], BF16, tag="cqt",
                                     name="cqT")
                    ssq_sb = p_stats.tile([128, MT], F32, tag="ssq",
                                          name="ssq_sb")
                    sq_tiles = {}
                    xt_tiles = {}

                    def emit_ssq(p):
                        ss_ps = p_ps.tile([128, 512], F32, tag="ps",
                                          name="ss_ps")[:, 0:MT]
                        for m in range(MT):
                            for ct in range(4):
                                nc.tensor.matmul(
                                    ss_ps[:, m:m + 1],
                                    lhsT=sq_tiles[p][ct][:, m * 128:
                                                         (m + 1) * 128],
                                    rhs=ones1,
                                    start=(ct == 0), stop=(ct == 3))
                        if p == 0:
                            nc.vector.tensor_copy(out=ssq_sb, in_=ss_ps)
                        else:
                            nc.vector.tensor_add(out=ssq_sb, in0=ssq_sb,
                                                 in1=ss_ps)

                    for p in range(3):
                        cb_ps = [p_ps.tile([128, 512], F32, tag="ps",
                                           name="ps") for _ in range(4)]
                        for k2 in range(KH2):
                            if p == 0:
                                xt_tiles[k2] = p_xt.tile(
                                    [128, 2, TC], BF16, tag=f"xt{k2}",
                                    name=f"xt{k2}")
                                if k2 == 0:
                                    for b in range(2):
                                        nc.sync.dma_start(
                                            out=xt_tiles[k2][:, b, :],
                                            in_=xt.ap()[b * 128:
                                                        (b + 1) * 128, :])
                                else:
                                    nc.sync.dma_start(
                                        out=xt_tiles[k2],
                                        in_=xt.ap()[k2 * 256:(k2 + 1) * 256,
                                                    :]
                                        .rearrange("(b p) t -> p b t", p=128))
                                if k2 == 0:
                                    g_ap = gkv.ap()
                                    nc.sync.dma_start(
                                        out=gamma_b,
                                        in_=bass.AP(
                                            tensor=g_ap.tensor,
                                            offset=g_ap.offset,
                                            ap=[[0, 128]] + [list(q)
                                                             for q in
                                                             g_ap.ap]))
                                    if out_bf16:
                                        nc.vector.tensor_copy(out=gamma_o,
                                                              in_=gamma_b)
                                    nc.sync.dma_start(
                                        out=cs_all,
                                        in_=cs.ap().rearrange(
                                            "(mt q) f -> q mt f", q=128))
                                    nc.sync.dma_start(
                                        out=cs_loc_t,
                                        in_=cs_loc.ap().rearrange(
                                            "(mt q) f -> q mt f", q=128))
                            wdq_t = p_wdq.tile([128, 2, 512], BF16,
                                               tag="wdq", name="wdq_t")
                            nc.sync.dma_start(
                                out=wdq_t,
                                in_=wdq.ap()[(p * KH2 + k2) * 128:
                                             (p * KH2 + k2 + 1) * 128, :]
                                .rearrange("q (b c) -> q b c", c=512))
                            for b in range(2):
                                k = 2 * k2 + b
                                for cb in range(4):
                                    nc.tensor.matmul(
                                        cb_ps[cb],
                                        lhsT=wdq_t[:, b,
                                                   cb * 128:(cb + 1) * 128],
                                        rhs=xt_tiles[k2][:, b, :],
                                        start=(k == 0), stop=(k == KH - 1))
                            if p > 0 and k2 == 2:
                                emit_ssq(p - 1)
                        sq_tiles[p] = []
                        for cb in range(4):
                            kq = p * 4 + cb
                            nc.vector.tensor_copy(
                                out=cqT[:, kq, :], in_=cb_ps[cb])
                            sq = p_sq.tile([128, 512], BF16, tag="sq",
                                           name="sq")
                            nc.scalar.activation(out=sq, in_=cb_ps[cb],
                                                 func=Sq)
                            sq_tiles[p].append(sq)

                    # ==== s4 + gather (collectives), kv overlaps ====
                    emit_ssq(2)
                    s4 = p_stats.tile([128, MT], F32, tag="s4", name="s4")
                    nc.scalar.activation(
                        out=s4, in_=ssq_sb, func=Sqrt,
                        bias=eps_t, scale=1.0 / L)
                    nc.vector.reciprocal(out=s4, in_=s4)
                    nc.sync.dma_start(out=s4b_in, in_=s4)
                    nc.sync.dma_start(out=cg_in, in_=cqT)
                    nc.gpsimd.collective_compute(
                        "AllGather", mybir.AluOpType.bypass,
                        replica_groups=[list(range(N_CORES))],
                        ins=[s4b_in[:]], outs=[s4b_out[:]])
                    nc.gpsimd.collective_compute(
                        "AllGather", mybir.AluOpType.bypass,
                        replica_groups=[list(range(N_CORES))],
                        ins=[cg_in[:]], outs=[cg_out[:]])

                    # kv latent + kropeT (PE busy while gather flies)
                    kv_ps = [p_ps.tile([128, 512], F32, tag="ps", name="ps")
                             for _ in range(MT)]
                    kr_ps = p_ps.tile([128, 512], F32, tag="ps", name="ps")
                    for k2 in range(KH2):
                        wkv_t = p_wdkv.tile([128, 2, 576], BF16, tag="wdkv",
                                            name="wkv_t")
                        nc.sync.dma_start(
                            out=wkv_t,
                            in_=wdkv.ap()[k2 * 128:(k2 + 1) * 128, :]
                            .rearrange("q (b c) -> q b c", c=576))
                        for b in range(2):
                            k = 2 * k2 + b
                            for m in range(MT):
                                nc.tensor.matmul(
                                    kv_ps[m],
                                    lhsT=xt_tiles[k2][:, b,
                                                      m * 128:(m + 1) * 128],
                                    rhs=wkv_t[:, b, 0:512],
                                    start=(k == 0), stop=(k == KH - 1))
                            nc.tensor.matmul(
                                kr_ps[0:64, :],
                                lhsT=wkv_t[:, b, 512:576],
                                rhs=xt_tiles[k2][:, b, :],
                                start=(k == 0), stop=(k == KH - 1))

                    # kv rmsnorm * gamma
                    kv_sb = []
                    for m in range(MT):
                        kv_m = p_kv.tile([128, KV_RANK + R], out_dt,
                                         tag=f"kv{m}", name=f"kv{m}")
                        st = p_stats.tile([128, 1], F32, tag=f"st{m}",
                                          name=f"st{m}")
                        scr = p_sq.tile([128, 512], BF16, tag="sq",
                                        name="scr")
                        nc.scalar.activation(
                            out=scr, in_=kv_ps[m], func=Sq, accum_out=st)
                        nc.scalar.activation(
                            out=st, in_=st, func=Sqrt,
                            bias=eps_t, scale=1.0 / KV_RANK)
                        nc.vector.reciprocal(out=st, in_=st)
                        nc.vector.tensor_scalar_mul(
                            out=kv_m[:, 0:KV_RANK], in0=kv_ps[m], scalar1=st)
                        nc.vector.tensor_mul(
                            out=kv_m[:, 0:KV_RANK], in0=kv_m[:, 0:KV_RANK],
                            in1=gamma_o)
                        kv_sb.append(kv_m)

                    # kropeT -> token-major, rope, kv store.  Local token
                    # tile m is global tile (will be placed by host).
                    krT = p_krT.tile([64, 512], F32, tag="krT", name="krT")
                    nc.vector.tensor_copy(out=krT, in_=kr_ps[0:64, :])
                    for m in range(MT):
                        tpk = p_ps.tile([128, 512], F32, tag="ps", name="ps")
                        nc.tensor.transpose(
                            tpk[:, 0:64], krT[:, m * 128:(m + 1) * 128], idf)
                        kv_m = kv_sb[m]
                        # rope with PLAIN cos/sin of LOCAL tokens: host
                        # passes full cs; local token tile row base depends
                        # on core — so kv rope needs per-core cs!  Handled
                        # via a second small input cs_loc below.
                        x1 = tpk[:, 0:32]
                        x2 = tpk[:, 32:64]
                        xx = tpk[:, 0:64]
                        cpair = cs_loc_sb[m][:, 0:64]
                        s1n = cs_loc_sb[m][:, 192:224]
                        s2 = cs_loc_sb[m][:, 288:320]
                        ta = p_tmp.tile([128, 192], F32, tag="ta", name="ta")
                        tb = p_tmp.tile([128, 192], F32, tag="tb", name="tb")
                        nc.vector.tensor_mul(out=ta[:, 0:64], in0=xx,
                                             in1=cpair)
                        nc.vector.tensor_mul(out=tb[:, 0:32], in0=x2,
                                             in1=s1n)
                        nc.vector.tensor_mul(out=tb[:, 32:64], in0=x1,
                                             in1=s2)
                        nc.vector.tensor_add(
                            out=kv_m[:, KV_RANK:KV_RANK + R],
                            in0=ta[:, 0:64], in1=tb[:, 0:64])
                        nc.sync.dma_start(
                            out=kvout.ap()[m * 128:(m + 1) * 128, :],
                            in_=kv_m)

                # ==== mm2: q[T, 3072] = gathered cq.T @ wb shard ====
                with (
                    tc.tile_pool(name="cqg", bufs=1) as p_cqg,
                    tc.tile_pool(name="wb", bufs=wb_bufs) as p_wb,
                ):
                    s4g = p_stats.tile([128, N_CORES, MT], F32, tag="s4g",
                                       name="s4g")
                    nc.sync.dma_start(
                        out=s4g,
                        in_=s4b_out[:].rearrange("(g q) f -> q g f", q=128))
                    cqTg = p_cqg.tile([128, KL, N_CORES, TC], BF16,
                                      tag="cqg", name="cqTg")
                    for d in range(N_CORES):
                        nc.sync.dma_start(
                            out=cqTg[:, :, d, :],
                            in_=cg_out[d * 128:(d + 1) * 128, :]
                            .rearrange("q (k t) -> q k t", t=TC))
                    for n in range(NTL):
                        pat = MM2_PAT[n % 3]
                        wb_tiles = []
                        for k2 in range(KL2):
                            wb_t = p_wb.tile([128, 2, NW], BF16, tag="wb",
                                             name="wb_t")
                            nc.sync.dma_start(
                                out=wb_t,
                                in_=wb.ap()[(n * KL2 + k2) * 128:
                                            (n * KL2 + k2 + 1) * 128, :]
                                .rearrange("q (b c) -> q b c", c=NW))
                            wb_tiles.append(wb_t)
                        for mg in range(N_CORES):
                            q_ps = [p_ps.tile([128, 512], F32, tag="ps",
                                              name="ps") for _ in range(MT)]
                            for k2 in range(KL2):
                                for b in range(2):
                                    k = 2 * k2 + b
                                    for m in range(MT):
                                        nc.tensor.matmul(
                                            q_ps[m],
                                            lhsT=cqTg[:, k, mg,
                                                      m * 128:(m + 1) * 128],
                                            rhs=wb_tiles[k2][:, b, :],
                                            start=(k == 0),
                                            stop=(k == KL - 1))
                            for m in range(MT):
                                mt = mg * MT + m
                                qo = p_qout.tile([128, NW], out_dt, tag="q",
                                                 name="qo")
                                qp = q_ps[m]
                                sc = s4g[:, mg, m:m + 1]
                                for base, nb, w in pat["nope"]:
                                    nc.vector.tensor_scalar_mul(
                                        out=_blocks(qo, base, nb, w),
                                        in0=_blocks(qp, base, nb, w),
                                        scalar1=sc)
                                rb, nr = pat["rope"]
                                x = _blocks(qp, rb, nr, 64)
                                x1 = _blocks(qp, rb, nr, 32)
                                x2 = _blocks(qp, rb + 32, nr, 32)
                                csm = cs_all[:, mt, :]
                                cpair = _blocks(csm, 0, nr, 64, stride=64)
                                s1n = _blocks(csm, 192, nr, 32, stride=32)
                                s2 = _blocks(csm, 288, nr, 32, stride=32)
                                ta = p_tmp.tile([128, 192], F32, tag="ta",
                                                name="ta")
                                tb = p_tmp.tile([128, 192], F32, tag="tb",
                                                name="tb")
                                va = _blocks(ta, 0, nr, 64, stride=64)
                                vb_lo = _blocks(tb, 0, nr, 32, stride=64)
                                vb_hi = _blocks(tb, 32, nr, 32, stride=64)
                                vb = _blocks(tb, 0, nr, 64, stride=64)
                                nc.vector.tensor_mul(out=va, in0=x,
                                                     in1=cpair)
                                nc.vector.tensor_mul(out=vb_lo, in0=x2,
                                                     in1=s1n)
                                nc.vector.tensor_mul(out=vb_hi, in0=x1,
                                                     in1=s2)
                                # rope is linear in x: scale by s4 after
                                nc.vector.tensor_add(out=va, in0=va, in1=vb)
                                nc.vector.tensor_scalar_mul(
                                    out=_blocks(qo, rb, nr, 64),
                                    in0=va, scalar1=sc)
                                nc.sync.dma_start(
                                    out=qout_ap[mt * 128:(mt + 1) * 128,
                                                n * NW:(n + 1) * NW],
                                    in_=qo)
    if split:
        split_multi_waits(nc)
    return nc


def prep_inputs(token_x, wq_a, wq_b, wkv, rope_cos, rope_sin, gamma_cq,
                gamma_ckv):
    """Host-side sharding + layout prep. Returns in_maps for the 8 cores."""
    bf16 = mybir.dt.np(BF16)
    # wkv -> per-k2 tiles [KH2, 128, 2, 576], flattened to 2D
    wdkv = (wkv.astype(np.float32).astype(bf16)
            .reshape(KH2, 2, 128, 576).transpose(0, 2, 1, 3)
            .reshape(KH2 * 128, 2 * 576))
    wdkv = np.ascontiguousarray(wdkv)
    # wq_a -> per (phase, k2) tiles [3, KH2, 128, 2, 512]
    wdq = (wq_a.astype(np.float32).astype(bf16)
           .reshape(KH2, 2, 128, 3, 512).transpose(3, 0, 2, 1, 4)
           .reshape(3 * KH2 * 128, 2 * 512))
    wdq = np.ascontiguousarray(wdq)
    # wq_b * gamma_cq -> per (n, k2) tiles [NT, KL2, 128, 2, 384], orig order
    wbs = wq_b.astype(np.float32) * gamma_cq.astype(np.float32)[:, None]
    wbt = (wbs.astype(bf16)
           .reshape(KL2, 2, 128, NT, NW).transpose(3, 0, 2, 1, 4)
           .reshape(NT * KL2 * 128, 2 * NW))
    wbt = np.ascontiguousarray(wbt)
    gkv = np.ascontiguousarray(gamma_ckv.astype(np.float32))
    cos = rope_cos.astype(np.float32)
    sin = rope_sin.astype(np.float32)
    c1, c2 = cos[:, 0:32], cos[:, 32:64]
    s1, s2 = sin[:, 0:32], sin[:, 32:64]
    # [cpair x3 | -s1 x3 | s2 x3]  (cpair = [c1|c2]) for the 4-op fused rope
    cs_full = np.concatenate([c1, c2, c1, c2, c1, c2,
                              -s1, -s1, -s1, s2, s2, s2], axis=1)
    in_maps = []
    for c in range(N_CORES):
        sl = slice(c * TC, (c + 1) * TC)
        xt = np.ascontiguousarray(token_x[sl].T).astype(bf16)      # [H, TC]
        cs = np.ascontiguousarray(cs_full[sl])                     # [TC, 256]
        in_maps.append({"xt": xt, "wdkv": wdkv, "wdq": wdq, "wb": wbt,
                        "cs": cs, "gkv": gkv})
    return in_maps


def prep_inputs_tp(token_x, wq_a, wq_b, wkv, rope_cos, rope_sin, gamma_cq,
                   gamma_ckv):
    """Host prep for the tensor-parallel kernel: wb sharded over 6-n-tile
    groups (16 heads/core); full cs replicated; rest as baseline."""
    bf16 = mybir.dt.np(BF16)
    NTL = NT // N_CORES
    wdkv = (wkv.astype(np.float32).astype(bf16)
            .reshape(KH2, 2, 128, 576).transpose(0, 2, 1, 3)
            .reshape(KH2 * 128, 2 * 576))
    wdkv = np.ascontiguousarray(wdkv)
    wdq = (wq_a.astype(np.float32).astype(bf16)
           .reshape(KH2, 2, 128, 3, 512).transpose(3, 0, 2, 1, 4)
           .reshape(3 * KH2 * 128, 2 * 512))
    wdq = np.ascontiguousarray(wdq)
    wbs = wq_b.astype(np.float32) * gamma_cq.astype(np.float32)[:, None]
    wbt = (wbs.astype(bf16)
           .reshape(KL2, 2, 128, NT, NW).transpose(3, 0, 2, 1, 4)
           .reshape(NT, KL2 * 128, 2 * NW))
    gkv = np.ascontiguousarray(gamma_ckv.astype(np.float32))
    cos = rope_cos.astype(np.float32)
    sin = rope_sin.astype(np.float32)
    c1, c2 = cos[:, 0:32], cos[:, 32:64]
    s1, s2 = sin[:, 0:32], sin[:, 32:64]
    cs_full = np.concatenate([c1, c2, c1, c2, c1, c2,
                              -s1, -s1, -s1, s2, s2, s2], axis=1)
    cs_full = np.ascontiguousarray(cs_full)
    in_maps = []
    for c in range(N_CORES):
        sl = slice(c * TC, (c + 1) * TC)
        xt = np.ascontiguousarray(token_x[sl].T).astype(bf16)
        wb_c = np.ascontiguousarray(
            wbt[c * NTL:(c + 1) * NTL].reshape(NTL * KL2 * 128, 2 * NW))
        in_maps.append({"xt": xt, "wdkv": wdkv, "wdq": wdq, "wb": wb_c,
                        "cs": cs_full, "cs_loc":
                        np.ascontiguousarray(cs_full[sl]), "gkv": gkv})
    return in_maps


def assemble_tp(results):
    """results: list of per-core {'qout': [T, 3072], 'kvout': [TC, 576]}."""
    NTL = NT // N_CORES
    out = np.empty((T, OUTW), np.float32)
    for c in range(N_CORES):
        out[:, c * NTL * NW:(c + 1) * NTL * NW] = (
            np.asarray(results[c]["qout"]).astype(np.float32))
        out[c * TC:(c + 1) * TC, DN:] = (
            np.asarray(results[c]["kvout"]).astype(np.float32))
    return out


def kernel(token_x, wq_a, wq_b, wkv, rope_cos, rope_sin, gamma_cq, gamma_ckv):
    token_x, wq_a, wq_b, wkv, rope_cos, rope_sin, gamma_cq, gamma_ckv = (
        np.asarray(a) for a in (token_x, wq_a, wq_b, wkv, rope_cos, rope_sin,
                                gamma_cq, gamma_ckv))
    tp = os.environ.get("MLA_TP", "0") == "1"
    if tp:
        in_maps = prep_inputs_tp(token_x, wq_a, wq_b, wkv, rope_cos,
                                 rope_sin, gamma_cq, gamma_ckv)
        nc = build_nc_tp()
        res = run_bass_kernel_spmd(nc, in_maps, list(range(N_CORES)))
        return assemble_tp(res.results)
    in_maps = prep_inputs(token_x, wq_a, wq_b, wkv, rope_cos, rope_sin,
                          gamma_cq, gamma_ckv)
    nc = build_nc()
    res = run_bass_kernel_spmd(nc, in_maps, list(range(N_CORES)))
    return np.concatenate(
        [np.asarray(res.results[c]["out"]).astype(np.float32)
         for c in range(N_CORES)], axis=0)

